# revision 4
# baseline (speedup 1.0000x reference)
"""NMI loss (soft-histogram mutual information) on 8 trn2 cores.

Voxel-sharded: each core handles N/8 = 262144 voxels as [128, 2048] f32.

Key identity: the Gaussian window exp(-preterm*(x-c)^2) is evaluated in a
SINGLE ACT pass per bin via Derivative_Erf:  erf'(t) = (2/sqrt(pi)) e^{-t^2}
with t = sqrt(preterm)*x + bias_i.  No subtract / square / exp stages, no
fp16 casts of x.  The (2/sqrt(pi)) factor cancels in all normalized terms;
only pab picks it up once, corrected on the host.

Per chunk of Vc=512 voxel-columns, per tensor:
  - ACT: 2 virtual-bin windows (centers -1/31, 32/31) into an i-major buf,
         32 real-bin windows written v-major (strided writes are free on
         ACT) into the Gram operand buffer J[v, 0..31], fp16.
  - DVE: S = sqrt(2) - Jv1 - Jv2 (Gaussian-comb identity in erf'-units; the
         2q*cos harmonic is ~1e-4 relative on the final MI and dropped).
         1/S via a degree-3 geometric series (z <= 0.11, err ~1e-4).
         A-side: J_a *= 1/S_a (v-major, 1x). B-side 1/S_b lands in the
         lhsT extra column.
  - PE : 512 contiguous [128,33]x[128,33] Gram pairs accumulating one
         33x33 PSUM:  out = [J_a/S_a | 1/S_b]^T @ [J_b | 1].

Host sums the 8 partial stats and does the tiny log-MI reduction exactly
as the reference (pab additionally scaled by sqrt(pi)/2).

Raw Bass blocks with manual semaphores, depth-2 buffers over chunks.
fp16 tensor_scalar needs f32 AP scalars (immediates mis-encode on HW).
"""

import sys
import numpy as np

sys.path.insert(0, "/opt/trn_rl_repo")

NCORES = 8
P = 128
B = 32                     # histogram bins
S = B + 1                  # Gram size (bins + marginal slot)
NVOX_TOTAL = 128 ** 3
NVOX = NVOX_TOTAL // NCORES
V = NVOX // P              # 2048 voxel-columns per partition
NCHUNK = 4
VC = V // NCHUNK           # 512

_BC = np.linspace(0.0, 1.0, B, dtype=np.float32)
_SIGMA = (np.mean(np.diff(_BC)) * np.float32(0.5)).astype(np.float32)
_PRETERM = (np.float32(1.0) / (np.float32(2.0) * _SIGMA * _SIGMA)).astype(np.float32)
_SCALE = float(np.sqrt(np.float64(_PRETERM)))     # sqrt(preterm)
_KERF = float(2.0 / np.sqrt(np.pi))               # erf' peak value
_CJ = float(np.sqrt(2.0))                         # comb sum in erf'-units

# consts tile (f32 [P, NC]):
#  col 0       : scale = sqrt(preterm)
#  cols 1..32  : -scale*c_i  (real bins)
#  col 33, 34  : -scale*c  for virtual centers -1/31, 32/31
#  cols 35..38 : series coeffs k3, k2, k1, k0   (1/S = k0+k1 s+k2 s^2+k3 s^3)
NCONST = 40


def _make_consts():
    c = np.zeros((P, NCONST), np.float32)
    c[:, 0] = _SCALE
    cen = np.concatenate([_BC.astype(np.float64), [-1.0 / 31.0, 32.0 / 31.0]])
    c[:, 1:35] = (-np.float64(_SCALE) * cen)[None, :].astype(np.float32)
    r = 1.0 / np.sqrt(2.0)
    c[:, 35] = r ** 4   # k3
    c[:, 36] = r ** 3   # k2
    c[:, 37] = r ** 2   # k1
    c[:, 38] = r        # k0
    return c


_CACHE = {}


def _build_nc():
    from contextlib import ExitStack
    from concourse import bass, mybir

    f32 = mybir.dt.float32
    f16 = mybir.dt.float16
    AF = mybir.ActivationFunctionType
    AL = mybir.AluOpType

    nc = bass.Bass()
    a_d = nc.dram_tensor("a", [P, V], f32, kind="ExternalInput")
    b_d = nc.dram_tensor("b", [P, V], f32, kind="ExternalInput")
    c_d = nc.dram_tensor("consts", [P, NCONST], f32, kind="ExternalInput")
    out_d = nc.dram_tensor("stats", [S, S], f32, kind="ExternalOutput")

    with ExitStack() as ctx:
        e = ctx.enter_context
        xa = e(nc.sbuf_tensor("xa", [P, V], f32))
        xb = e(nc.sbuf_tensor("xb", [P, V], f32))
        cst = e(nc.sbuf_tensor("cst", [P, NCONST], f32))
        ja = [e(nc.sbuf_tensor(f"ja{k}", [P, S * VC], f16)) for k in range(2)]
        jb = [e(nc.sbuf_tensor(f"jb{k}", [P, S * VC], f16)) for k in range(2)]
        va = [e(nc.sbuf_tensor(f"va{k}", [P, 2 * VC], f16)) for k in range(2)]
        vb = [e(nc.sbuf_tensor(f"vb{k}", [P, 2 * VC], f16)) for k in range(2)]
        s1 = e(nc.sbuf_tensor("s1", [P, VC], f16))
        h1 = e(nc.sbuf_tensor("h1", [P, VC], f16))
        h2 = e(nc.sbuf_tensor("h2", [P, VC], f16))
        rsa = [e(nc.sbuf_tensor(f"rsa{k}", [P, VC], f16)) for k in range(2)]
        rsb = [e(nc.sbuf_tensor(f"rsb{k}", [P, VC], f16)) for k in range(2)]
        stats_sb = e(nc.sbuf_tensor("stats_sb", [S, S], f32))
        acc = e(nc.psum_tensor("acc", [S, S], f32))

        s_dma = e(nc.semaphore("s_dma"))
        s_ones = e(nc.semaphore("s_ones"))
        s_va = e(nc.semaphore("s_va"))
        s_vb = e(nc.semaphore("s_vb"))
        s_ma = e(nc.semaphore("s_ma"))
        s_mb = e(nc.semaphore("s_mb"))
        s_mul = e(nc.semaphore("s_mul"))
        s_pe = e(nc.semaphore("s_pe"))
        s_done = e(nc.semaphore("s_done"))
        s_out = e(nc.semaphore("s_out"))
        block = e(nc.Block())

        def vslot(buf, i):
            # v-major strided AP: slot i of each voxel, [P, VC, 1]
            return buf[:, :].rearrange("p (v i) -> p v i", i=S)[:, :, i : i + 1]

        def xs(x, c):
            return x[:, c * VC : (c + 1) * VC]

        @block.sync
        def _(sync):
            sync.dma_start(cst[:, :], c_d[:, :]).then_inc(s_dma, 16)
            sync.dma_start(xa[:, :], a_d[:, :]).then_inc(s_dma, 16)
            sync.dma_start(xb[:, :], b_d[:, :]).then_inc(s_dma, 16)

        @block.scalar
        def _(scalar):
            scalar.wait_ge(s_dma, 48)
            for c in range(NCHUNK):
                k = c % 2
                if c >= 2:
                    scalar.wait_ge(s_pe, c - 1)
                for (jbuf, vbuf, x, s_v, s_m) in (
                    (ja[k], va[k], xa, s_va, s_ma),
                    (jb[k], vb[k], xb, s_vb, s_mb),
                ):
                    # virtual bins first so DVE's series overlaps main bins
                    for t in range(2):
                        ins = scalar.activation(
                            vbuf[:, t * VC : (t + 1) * VC], xs(x, c),
                            AF.Derivative_Erf,
                            bias=cst[:, 33 + t : 34 + t], scale=cst[:, 0:1],
                        )
                    ins.then_inc(s_v, 1)
                    for i in range(B):
                        ins = scalar.activation(
                            vslot(jbuf, i), xs(x, c), AF.Derivative_Erf,
                            bias=cst[:, 1 + i : 2 + i], scale=cst[:, 0:1],
                        )
                    ins.then_inc(s_m, 1)

        @block.vector
        def _(vector):
            for c in range(NCHUNK):
                k = c % 2
                for (vbuf, rs, s_v, c_add) in (
                    (va[k], rsa[k], s_va, 0),
                    (vb[k], rsb[k], s_vb, 0),
                ):
                    vector.wait_ge(s_v, c + 1)
                    vector.tensor_add(
                        s1[:, :], vbuf[:, 0:VC], vbuf[:, VC : 2 * VC]
                    )
                    # 1/S = ((k3*s1 + k2)*s1 + k1)*s1 + k0
                    vector.tensor_scalar(
                        h1[:, :], s1[:, :], cst[:, 35:36], cst[:, 36:37],
                        AL.mult, AL.add,
                    )
                    vector.tensor_mul(h2[:, :], h1[:, :], s1[:, :])
                    vector.tensor_scalar(
                        h1[:, :], h2[:, :], cst[:, 37:38], None, AL.add
                    )
                    vector.tensor_mul(h2[:, :], h1[:, :], s1[:, :])
                    vector.tensor_scalar(
                        rs[:, :], h2[:, :], cst[:, 38:39], None, AL.add
                    )
                # 1/S_b into lhsT extra column (slot 32 of A-buffer)
                vector.tensor_copy(vslot(ja[k], B), rsb[k][:, :])
                # normalize A: J_a *= 1/S_a over slots 0..31 (v-major, 1x)
                vector.wait_ge(s_ma, c + 1)
                av = ja[k][:, :].rearrange("p (v i) -> p v i", i=S)[:, :, 0:B]
                vector.tensor_mul(
                    av,
                    av,
                    rsa[k][:, :]
                    .rearrange("p (v o) -> p v o", o=1)
                    .broadcast_to([P, VC, B]),
                ).then_inc(s_mul, 1)

            vector.wait_ge(s_pe, NCHUNK)
            vector.tensor_copy(stats_sb[:, :], acc[:, :]).then_inc(s_done, 1)

        @block.tensor
        def _(tensor):
            tensor.wait_ge(s_ones, 2)
            for c in range(NCHUNK):
                k = c % 2
                tensor.wait_ge(s_mul, c + 1)
                tensor.wait_ge(s_mb, c + 1)
                lv = ja[k][:, :].rearrange("p (v i) -> p v i", i=S)
                rv = jb[k][:, :].rearrange("p (v i) -> p v i", i=S)
                for v in range(VC):
                    first = c == 0 and v == 0
                    last = c == NCHUNK - 1 and v == VC - 1
                    mm = tensor.matmul(
                        acc[:, :],
                        lv[:, v, :],
                        rv[:, v, :],
                        start=first,
                        stop=last,
                    )
                    if v == VC - 1:
                        mm.then_inc(s_pe, 1)

        @block.gpsimd
        def _(gpsimd):
            for k in range(2):
                gpsimd.memset(vslot(jb[k], B), 1.0).then_inc(s_ones, 1)
            gpsimd.wait_ge(s_done, 1)
            gpsimd.dma_start(out_d[:, :], stats_sb[:, :]).then_inc(s_out, 16)
            gpsimd.wait_ge(s_out, 16)

    return nc


def _get_nc():
    if "nc" not in _CACHE:
        _CACHE["nc"] = _build_nc()
    return _CACHE["nc"]


def run_device(a_flat, b_flat, trace=False):
    """Run the per-core bass kernel on 8 cores; returns (stats_sum, results)."""
    from concourse.bass_utils import run_bass_kernel_spmd

    nc = _get_nc()
    consts = _make_consts()
    a3 = a_flat.reshape(NCORES, P, V)
    b3 = b_flat.reshape(NCORES, P, V)
    in_maps = [
        {"a": np.ascontiguousarray(a3[i]), "b": np.ascontiguousarray(b3[i]),
         "consts": consts}
        for i in range(NCORES)
    ]
    kw = {}
    if trace:
        kw.update(trace=True, trace_cores=[0])
    res = run_bass_kernel_spmd(nc, in_maps, list(range(NCORES)), **kw)
    stats = np.zeros((S, S), np.float64)
    for r in res.results:
        stats += np.asarray(r["stats"], np.float64)
    return stats, res


def finish(stats):
    n = float(NVOX_TOTAL)
    # J carries a 2/sqrt(pi) factor vs the reference windows. It cancels in
    # pa (A side normalized) and pb (1/S_b in erf'-units), but pab's rhs is
    # the raw J_b: divide by 2/sqrt(pi) once.
    pab = stats[0:B, 0:B] / (n * _KERF)
    pa = stats[0:B, B] / n
    pb = stats[B, 0:B] / n
    eps = 1.4e-45
    papb = np.outer(pa, pb) + eps
    mi = np.sum(pab * np.log(pab / papb + eps))
    return np.array([-mi], dtype=np.float32)


def kernel(actual, target):
    a = np.clip(np.asarray(actual, np.float32).reshape(-1), 0.0, 1.0)
    b = np.clip(np.asarray(target, np.float32).reshape(-1), 0.0, 1.0)
    stats, _ = run_device(a, b)
    return finish(stats)


# revision 7
# speedup vs baseline: 2.2582x; 2.2582x over previous
"""NMI loss (soft-histogram mutual information) on 8 trn2 cores.

Each core handles N/8 = 262144 voxels as [128, 2048] f32.

Gaussian windows via a single ACT pass per bin using Derivative_Erf:
  erf'(t) = (2/sqrt(pi)) e^{-t^2},  t = sqrt(preterm)*x - sqrt(preterm)*c_i
(no subtract/square/exp stages). Two extra "virtual" bins at centers
-1/31 and 32/31 give the per-voxel normalizer analytically via the
Gaussian-comb identity  S = sqrt(2) - J[-1] - J[32]  (in erf'-units; the
2q*cos harmonic is ~1e-4 relative on the final MI and dropped). 1/S via
a degree-3 geometric series on DVE (z <= 0.11).

Gram structure (per voxel-column v):  out += lhsT_v^T @ rhs_v with
  lhsT_v[33] = [J_b*rS_a | rS_a]    (v-major fp16, built by ONE fused
                                     multiply+transpose DVE pass whose
                                     input block 32 is constant 1.0)
  rhs_v[33]  = [J_a | S_a/S_b]      (i-major fp16, raw ACT output)
PE consumes voxel PAIRS: lhsT [128,66] contiguous, rhs [128,33,2]
strided -> 50 ns/pair (the stride penalty hides under the 66-col
weight load). PSUM is [66,66]; host-relevant blocks are extracted once
at the end. All 2/sqrt(pi) factors cancel on-device except one k on
pab, corrected on the host:
  out[j,i] = k*n*pab[i,j], out[32,i] = n*pa[i], out[j,32] = n*pb[j].

Raw Bass blocks, manual semaphores, depth-2 buffers (jB single-buffered,
its const block persists). fp16 tensor_scalar needs f32 AP scalars.
"""

import sys
import numpy as np

sys.path.insert(0, "/opt/trn_rl_repo")

NCORES = 8
P = 128
B = 32
S = B + 1                  # 33 Gram slots
NVOX_TOTAL = 128 ** 3
NVOX = NVOX_TOTAL // NCORES
V = NVOX // P              # 2048
NCHUNK = 4
VC = V // NCHUNK           # 512
NG = VC // 2               # 256 voxel-pair matmuls per chunk

_BC = np.linspace(0.0, 1.0, B, dtype=np.float32)
_SIGMA = (np.mean(np.diff(_BC)) * np.float32(0.5)).astype(np.float32)
_PRETERM = (np.float32(1.0) / (np.float32(2.0) * _SIGMA * _SIGMA)).astype(np.float32)
_SCALE = float(np.sqrt(np.float64(_PRETERM)))
_KERF = float(2.0 / np.sqrt(np.pi))

# consts (f32 [P, NC]): 0: scale; 1..32: -scale*c_i; 33,34: virtual biases;
# 35..38: series k3,k2,k1,k0; 39: -1.0; 40: sqrt(2)
NCONST = 44


def _make_consts():
    c = np.zeros((P, NCONST), np.float32)
    c[:, 0] = _SCALE
    cen = np.concatenate([_BC.astype(np.float64), [-1.0 / 31.0, 32.0 / 31.0]])
    c[:, 1:35] = (-np.float64(_SCALE) * cen)[None, :].astype(np.float32)
    r = 1.0 / np.sqrt(2.0)
    c[:, 35] = r ** 4
    c[:, 36] = r ** 3
    c[:, 37] = r ** 2
    c[:, 38] = r
    c[:, 39] = -1.0
    c[:, 40] = np.sqrt(2.0)
    return c


_CACHE = {}


def _build_nc():
    from contextlib import ExitStack
    from concourse import bass, mybir

    f32 = mybir.dt.float32
    f16 = mybir.dt.float16
    AF = mybir.ActivationFunctionType
    AL = mybir.AluOpType

    nc = bass.Bass()
    a_d = nc.dram_tensor("a", [P, V], f32, kind="ExternalInput")
    b_d = nc.dram_tensor("b", [P, V], f32, kind="ExternalInput")
    c_d = nc.dram_tensor("consts", [P, NCONST], f32, kind="ExternalInput")
    out_d = nc.dram_tensor("stats", [2 * S, 2 * S], f32, kind="ExternalOutput")

    with ExitStack() as ctx:
        e = ctx.enter_context
        xa = e(nc.sbuf_tensor("xa", [P, V], f32))
        xb = e(nc.sbuf_tensor("xb", [P, V], f32))
        cst = e(nc.sbuf_tensor("cst", [P, NCONST], f32))
        # i-major: block i at offset i*VC (33 blocks; jb block 32 = const 1)
        jA = [e(nc.sbuf_tensor(f"jA{k}", [P, S * VC], f16)) for k in range(2)]
        jB = e(nc.sbuf_tensor("jB", [P, S * VC], f16))
        # v-major lhsT: voxel v slots at v*33..v*33+32
        bv = [e(nc.sbuf_tensor(f"bv{k}", [P, S * VC], f16)) for k in range(2)]
        va = [e(nc.sbuf_tensor(f"va{k}", [P, 2 * VC], f16)) for k in range(2)]
        vb = [e(nc.sbuf_tensor(f"vb{k}", [P, 2 * VC], f16)) for k in range(2)]
        s1a = e(nc.sbuf_tensor("s1a", [P, VC], f16))
        s1b = e(nc.sbuf_tensor("s1b", [P, VC], f16))
        h1 = e(nc.sbuf_tensor("h1", [P, VC], f16))
        h2 = e(nc.sbuf_tensor("h2", [P, VC], f16))
        rsa = e(nc.sbuf_tensor("rsa", [P, VC], f16))
        rsb = e(nc.sbuf_tensor("rsb", [P, VC], f16))
        sat = e(nc.sbuf_tensor("sat", [P, VC], f16))
        stats_sb = e(nc.sbuf_tensor("stats_sb", [2 * S, 2 * S], f32))
        acc = e(nc.psum_tensor("acc", [2 * S, 2 * S], f32))

        s_dma = e(nc.semaphore("s_dma"))
        s_ones = e(nc.semaphore("s_ones"))
        s_va = e(nc.semaphore("s_va"))
        s_vb = e(nc.semaphore("s_vb"))
        s_ma = e(nc.semaphore("s_ma"))
        s_mb = e(nc.semaphore("s_mb"))
        s_mul = e(nc.semaphore("s_mul"))
        s_pe = e(nc.semaphore("s_pe"))
        s_done = e(nc.semaphore("s_done"))
        s_out = e(nc.semaphore("s_out"))
        block = e(nc.Block())

        def blk(buf, i):
            return buf[:, i * VC : (i + 1) * VC]

        def xs(x, c):
            return x[:, c * VC : (c + 1) * VC]

        @block.sync
        def _(sync):
            sync.dma_start(cst[:, :], c_d[:, :]).then_inc(s_dma, 16)
            sync.dma_start(xa[:, :], a_d[:, :]).then_inc(s_dma, 16)
            sync.dma_start(xb[:, :], b_d[:, :]).then_inc(s_dma, 16)

        @block.scalar
        def _(scalar):
            scalar.wait_ge(s_dma, 48)
            for c in range(NCHUNK):
                k = c % 2
                if c >= 2:
                    scalar.wait_ge(s_pe, c - 1)
                for (jbuf, vbuf, x, s_v, s_m) in (
                    (jA[k], va[k], xa, s_va, s_ma),
                    (jB, vb[k], xb, s_vb, s_mb),
                ):
                    if jbuf is jB and c >= 1:
                        scalar.wait_ge(s_mul, c)  # jB single-buffered
                    for t in range(2):
                        ins = scalar.activation(
                            vbuf[:, t * VC : (t + 1) * VC], xs(x, c),
                            AF.Derivative_Erf,
                            bias=cst[:, 33 + t : 34 + t], scale=cst[:, 0:1],
                        )
                    ins.then_inc(s_v, 1)
                    for i in range(B):
                        ins = scalar.activation(
                            blk(jbuf, i), xs(x, c), AF.Derivative_Erf,
                            bias=cst[:, 1 + i : 2 + i], scale=cst[:, 0:1],
                        )
                    ins.then_inc(s_m, 1)

        @block.vector
        def _(vector):
            for c in range(NCHUNK):
                k = c % 2
                if c >= 2:
                    vector.wait_ge(s_pe, c - 1)
                # 1/S series for both tensors
                for (vbuf, rs, s_v) in ((va[k], rsa, s_va), (vb[k], rsb, s_vb)):
                    vector.wait_ge(s_v, c + 1)
                    sdst = s1a if rs is rsa else s1b
                    vector.tensor_add(
                        sdst[:, :], vbuf[:, 0:VC], vbuf[:, VC : 2 * VC]
                    )
                    vector.tensor_scalar(
                        h1[:, :], sdst[:, :], cst[:, 35:36], cst[:, 36:37],
                        AL.mult, AL.add,
                    )
                    vector.tensor_mul(h2[:, :], h1[:, :], sdst[:, :])
                    vector.tensor_scalar(
                        h1[:, :], h2[:, :], cst[:, 37:38], None, AL.add
                    )
                    vector.tensor_mul(h2[:, :], h1[:, :], sdst[:, :])
                    vector.tensor_scalar(
                        rs[:, :], h2[:, :], cst[:, 38:39], None, AL.add
                    )
                # S_a = sqrt(2) - s1a;  rhs extra col Y = S_a * rS_b -> jA blk 32
                vector.tensor_scalar(
                    sat[:, :], s1a[:, :], cst[:, 39:40], cst[:, 40:41],
                    AL.mult, AL.add,
                )
                vector.tensor_mul(blk(jA[k], B), sat[:, :], rsb[:, :])
                # fused normalize+transpose: bv[v*33+i] = jB[i*VC+v] * rS_a[v]
                vector.wait_ge(s_mb, c + 1)
                if c == 0:
                    vector.wait_ge(s_ones, 1)
                dv = bv[k][:, :].rearrange("p (v i) -> p v i", i=S)
                sv = jB[:, :].rearrange("p (i v) -> p v i", v=VC)
                vector.tensor_mul(
                    dv, sv,
                    rsa[:, :]
                    .rearrange("p (v o) -> p v o", o=1)
                    .broadcast_to([P, VC, S]),
                ).then_inc(s_mul, 1)

            vector.wait_ge(s_pe, NCHUNK)
            vector.tensor_copy(stats_sb[:, :], acc[:, :]).then_inc(s_done, 1)

        @block.tensor
        def _(tensor):
            for c in range(NCHUNK):
                k = c % 2
                tensor.wait_ge(s_mul, c + 1)
                tensor.wait_ge(s_ma, c + 1)
                lv = bv[k][:, :].rearrange("p (vg ui) -> p vg ui", ui=2 * S)
                rv = jA[k][:, :].rearrange("p (n vg u) -> p n vg u", n=S, u=2)
                for vg in range(NG):
                    first = c == 0 and vg == 0
                    last = c == NCHUNK - 1 and vg == NG - 1
                    mm = tensor.matmul(
                        acc[:, :],
                        lv[:, vg, :],
                        rv[:, :, vg, :],
                        start=first,
                        stop=last,
                    )
                    if vg == NG - 1:
                        mm.then_inc(s_pe, 1)

        @block.gpsimd
        def _(gpsimd):
            gpsimd.memset(blk(jB, B), 1.0).then_inc(s_ones, 1)
            gpsimd.wait_ge(s_done, 1)
            gpsimd.dma_start(out_d[:, :], stats_sb[:, :]).then_inc(s_out, 16)
            gpsimd.wait_ge(s_out, 16)

    return nc


def _get_nc():
    if "nc" not in _CACHE:
        _CACHE["nc"] = _build_nc()
    return _CACHE["nc"]


def run_device(a_flat, b_flat, trace=False):
    from concourse.bass_utils import run_bass_kernel_spmd

    nc = _get_nc()
    consts = _make_consts()
    a3 = a_flat.reshape(NCORES, P, V)
    b3 = b_flat.reshape(NCORES, P, V)
    in_maps = [
        {"a": np.ascontiguousarray(a3[i]), "b": np.ascontiguousarray(b3[i]),
         "consts": consts}
        for i in range(NCORES)
    ]
    kw = {}
    if trace:
        kw.update(trace=True, trace_cores=[0])
    res = run_bass_kernel_spmd(nc, in_maps, list(range(NCORES)), **kw)
    stats = np.zeros((S, S), np.float64)
    for r in res.results:
        s66 = np.asarray(r["stats"], np.float64)
        stats += s66[0:S, 0::2] + s66[S : 2 * S, 1::2]
    return stats, res


def finish(stats):
    n = float(NVOX_TOTAL)
    # stats[j, i] = k*n*pab[i, j]; stats[32, i] = n*pa[i]; stats[j, 32] = n*pb[j]
    pab = stats[0:B, 0:B].T / (n * _KERF)
    pa = stats[B, 0:B] / n
    pb = stats[0:B, B] / n
    eps = 1.4e-45
    papb = np.outer(pa, pb) + eps
    mi = np.sum(pab * np.log(pab / papb + eps))
    return np.array([-mi], dtype=np.float32)


def kernel(actual, target):
    a = np.clip(np.asarray(actual, np.float32).reshape(-1), 0.0, 1.0)
    b = np.clip(np.asarray(target, np.float32).reshape(-1), 0.0, 1.0)
    stats, _ = run_device(a, b)
    return finish(stats)
